# revision 31
# baseline (speedup 1.0000x reference)
"""Trainium2 Bass kernel for nn_BDFM_Multi (B=8,C=256,H=W=128,N=4).

Data-parallel over batch: one batch element per NeuronCore (8 cores).

Per-core computation (feature f [C,HW], m [N,H,W], HW=16384):
  z    = (m > 0.3)                                  binary
  er   = 13-tap separable min-filter(z), dl = 13-tap separable max-filter(z)
         (composition of 4 iters of 4x4 cv2-style erode/dilate)
         -> computed as banded 0/1 matmuls + thresholds (exact on binary data)
  fbu  = per-class channels (er, 1-dl, dl-er)       [12, HW]
  mid  = fbu @ f^T                                  [12, C]
  A'   = Wo2' @ mid^T                               [C, 12]
  G    = A' @ mid                                   [C, C]
  Wc   = Wo1' + G @ Wf'                             [C, C]  <- key collapse:
         out = Wo1'@f + G@(Wf'@f + beta_f 1^T) + beta_o 1^T
             = Wc @ f + u 1^T,   u = G @ beta_f + beta_o
  out  = Wc @ f + u                                 [C, HW]
  (exact algebraic refactor of out = BN(Wo @ [f; mid^T @ (mid @ BN(Wf@f))]))

Schedule notes (the kernel is HBM-bound: 16MB in + 16MB out per core with a
true barrier between them, since every output element depends on all of mid):
  - the whole feature load rides the SP (sync) HWDGE queue, which is alive
    ~6us before the SWDGE path; blk0/blk1 chunk pairs are interleaved with
    small chunks first (early pass-1 start) and last (small tail residual).
  - band/identity constants are generated on-chip (affine_select), not DMA'd.
  - pass 1 (PE transposes + mid accumulation) is software-pipelined and its
    PSUM evictions are split DVE/Act per half-group so it tracks DMA rate.
  - pass 2 is store-bound; PSUM tiles per output-row-block are evicted in
    1024-col ops (DVE for rows 0:128, Act for 128:256) and stored per 1024
    cols on the SP + gpsimd queues.

All big matmuls run in float32r (~2-4e-4 rel err, full PE rate).
"""
import numpy as np
from contextlib import ExitStack

import concourse.bass as bass
import concourse.mybir as mybir
import concourse.tile as tile
from concourse import bacc
from concourse import bass_utils
from concourse.masks import make_identity

F32 = mybir.dt.float32
F32R = mybir.dt.float32r
ALU = mybir.AluOpType
ACTF = mybir.ActivationFunctionType

B, C, H, W, N = 8, 256, 128, 128, 4
HW = H * W
EPS = 1e-5
P = 128

# feature load chunk sizes (cols per block).  Few, large chunks keep the
# queues at full bandwidth (each dma_start boundary costs a gap); the two
# queues land each blk0/blk1 pair concurrently, and the p-state filler
# matmuls bridge the 4-group visibility quantum of a 2048-col chunk.
# 512-col chunks at the head (fast pass-1 start) and tail (small residual).
CHUNKS = [512, 512, 1024, 2048, 4096, 4096, 2048, 1024, 512, 512]
assert sum(CHUNKS) == HW

_NC_CACHE = {}


def _band(nc, t, lo, hi):
    # t[i, m] = 1.0 iff m - lo <= i <= m + hi  (both affine_selects keep
    # where the affine expr passes, else fill 0)
    nc.gpsimd.memset(t, 1.0)
    nc.gpsimd.affine_select(out=t, in_=t, compare_op=ALU.is_ge, fill=0.0,
                            base=lo, pattern=[[-1, P]], channel_multiplier=1)
    # i <= m + hi  expressed as  (-i + m + hi) >= 0  (is_le unimplemented)
    nc.gpsimd.affine_select(out=t, in_=t, compare_op=ALU.is_ge, fill=0.0,
                            base=hi, pattern=[[1, P]], channel_multiplier=-1)


def build():
    if "nc" in _NC_CACHE:
        return _NC_CACHE["nc"]
    nc = bacc.Bacc(trn_type="TRN2", target_bir_lowering=False, debug=False)

    feature = nc.dram_tensor("feature", [C, HW], F32R, kind="ExternalInput")
    pkr = nc.dram_tensor("pkr", [P, 1536], F32R, kind="ExternalInput")
    pkf = nc.dram_tensor("pkf", [P, 517], F32, kind="ExternalInput")
    out = nc.dram_tensor("out", [C, HW], F32, kind="ExternalOutput")

    with tile.TileContext(nc) as tc, ExitStack() as ctx:
        persist = ctx.enter_context(tc.tile_pool(name="persist", bufs=1))

        feat = persist.tile([P, 2 * HW], F32R)     # c-blk0 cols | c-blk1 cols
        pkr_sb = persist.tile([P, 1536], F32R)
        pkf_sb = persist.tile([P, 517], F32)
        ident = persist.tile([P, P], F32R)
        band_er = persist.tile([P, P], F32R)
        band_dl = persist.tile([P, P], F32R)
        cgen = persist.tile([P, 3 * P], F32)       # Pool-written const scratch
        fbuT = persist.tile([P, 12 * P], F32R)     # [w, h*12 + k]
        wc_sb = persist.tile([P, 512], F32R)       # Wc^T blocks (a*2+o)
        u_sb = persist.tile([P, 2], F32)
        warm = persist.tile([P, 512], F32)
        warmb = persist.tile([P, 256], mybir.dt.bfloat16)  # p-state filler
        z_sb = persist.tile([P, N * P], F32R)

        wo2_sb = pkr_sb[:, 0:512]
        wfn_sb = pkr_sb[:, 512:1024]
        wo1_sb = pkr_sb[:, 1024:1536]
        m_sb = pkf_sb[:, 0:512]
        betao_sb = pkf_sb[:, 512:514]
        betaf_sb = pkf_sb[:, 514:516]
        cnt_sb = pkf_sb[:, 516:517]

        # ---------------- loads: everything on the SP HWDGE queue ----------
        # feature/pkr DRAM tensors are declared float32r (same 4-byte IEEE
        # bits; the PE rounds internally), so these DMAs are cast-free and
        # can ride the SP HWDGE queue, alive ~6us before the SWDGE path.
        # ---------------- on-chip constants --------------------------------
        # Pool affine_select writes fp32 scratch (Pool cannot write fp32r);
        # DVE copies perform the fp32->fp32r rounding cast.
        make_identity(nc, cgen[:, 0:P])
        # erosion: output i covers input [i-8, i+4]; band_er[i,m]=1 iff
        # m-8 <= i <= m+4.  dilation: [i-4, i+8].
        _band(nc, cgen[:, P:2 * P], 8, 4)
        _band(nc, cgen[:, 2 * P:3 * P], 4, 8)
        nc.vector.tensor_copy(ident[:], cgen[:, 0:P])
        nc.vector.tensor_copy(band_er[:], cgen[:, P:2 * P])
        nc.vector.tensor_copy(band_dl[:], cgen[:, 2 * P:3 * P])

        # single SP HWDGE queue: measured ~383 GB/s; multi-queue splits and
        # many small dma_starts both measured slower.
        nc.sync.dma_start(out=pkf_sb[:], in_=pkf[:])
        c0 = 0
        for i, w in enumerate(CHUNKS):
            c1 = c0 + w
            nc.sync.dma_start(out=feat[:, c0:c1], in_=feature[0:P, c0:c1])
            nc.sync.dma_start(out=feat[:, HW + c0:HW + c1],
                              in_=feature[P:C, c0:c1])
            c0 = c1
        # params for the small stage; needed only at ~load end
        nc.sync.dma_start(out=pkr_sb[:], in_=pkr[:])

        # ---------------- warmup + morphology ------------------------------
        with tc.tile_pool(name="warm_ps", bufs=1, space="PSUM") as wps:
            nc.vector.memset(warm[:], 0.0)
            nc.vector.memset(warmb[:], 0.0)
            # trigger the one-time ACT table load early (off critical path)
            nc.scalar.copy(u_sb[:, 0:1], warm[:, 0:1])
            wp = wps.tile([P, 512], F32)
            for _ in range(2):
                nc.tensor.matmul(wp[:, 0:P], warm[:, 0:P], warm[:, 0:P],
                                 start=True, stop=True, skip_group_check=True)

        # ---------------- morphology + pass 1, interleaved ------------------
        # Engines execute in issue order, so morphology ops are interleaved
        # into the first pass-1 groups: otherwise the early pass-1 PSUM
        # evictions queue behind the whole morph chain on DVE/Act, the
        # 2-buffer transpose pool backs up, and the (in-order) PE stalls.
        # fbu is built as three contiguous [128,512] planes (er|bg|maskd) in
        # one tile; the mid matmul reads a strided [w, n, j] lhsT AP.
        fill_ctx = ExitStack()
        fillps = fill_ctx.enter_context(
            tc.tile_pool(name="fill_ps", bufs=1, space="PSUM"))
        fill_ps = fillps.tile([P, 256], F32)

        def filler(k):
            for _ in range(k):
                nc.tensor.matmul(fill_ps[:], warmb[:, 0:P], warmb[:],
                                 start=True, stop=True,
                                 skip_group_check=True)

        morph_ctx = ExitStack()
        mo = morph_ctx.enter_context(tc.tile_pool(name="morph", bufs=1))
        mops = morph_ctx.enter_context(tc.tile_pool(name="morph_ps", bufs=2,
                                                    space="PSUM"))
        fbu3 = persist.tile([P, 3 * 512], F32R)    # col = (n*3+j)*128 + h
        dl_t = persist.tile([P, 512], F32)

        nc.vector.tensor_scalar(z_sb[:], m_sb, 0.3, None, op0=ALU.is_gt)
        ps_rows_er = mops.tile([P, N * P], F32, tag="mps")
        nc.tensor.matmul(ps_rows_er[:], band_er[:], z_sb[:],
                         start=True, stop=True)
        ps_rows_dl = mops.tile([P, N * P], F32, tag="mps")
        nc.tensor.matmul(ps_rows_dl[:], band_dl[:], z_sb[:],
                         start=True, stop=True)
        rows_er = mo.tile([P, N * P], F32R)
        nc.vector.tensor_scalar(rows_er[:], ps_rows_er[:], cnt_sb, None,
                                op0=ALU.is_equal)
        rows_dl = mo.tile([P, N * P], F32R)
        nc.vector.tensor_scalar(rows_dl[:], ps_rows_dl[:], 0.5, None,
                                op0=ALU.is_gt)
        rows_erT = mo.tile([P, N * P], F32R)
        rows_dlT = mo.tile([P, N * P], F32R)

        def morph_stage_0():
            # transpose each class tile -> [w, h]; one big eviction per kind
            ps_trs = []
            for n in range(N):
                ps_tr = mops.tile([P, 2 * P], F32R, tag="mps", name=f"ps_tr{n}")
                nc.tensor.matmul(ps_tr[:, 0:P], rows_er[:, n * P:(n + 1) * P],
                                 ident[:], is_transpose=True)
                nc.tensor.matmul(ps_tr[:, P:2 * P],
                                 rows_dl[:, n * P:(n + 1) * P],
                                 ident[:], is_transpose=True)
                ps_trs.append(ps_tr)
            for n, ps_tr in enumerate(ps_trs):
                nc.scalar.copy(rows_erT[:, n * P:(n + 1) * P], ps_tr[:, 0:P])
                nc.scalar.copy(rows_dlT[:, n * P:(n + 1) * P], ps_tr[:, P:2 * P])

        # fbu3 channel-plane views: v4[:, j] = [w, n, h] (k = n*3 + j)
        fbu_v4 = fbu3.rearrange("w (n j h) -> w j n h", n=4, j=3)
        fbu_v12 = fbu3.rearrange("w (k h) -> w k h", k=12)

        def morph_stage_1():
            ps_cols_er = mops.tile([P, N * P], F32, tag="mps")
            nc.tensor.matmul(ps_cols_er[:], band_er[:], rows_erT[:],
                             start=True, stop=True)
            ps_cols_dl = mops.tile([P, N * P], F32, tag="mps")
            nc.tensor.matmul(ps_cols_dl[:], band_dl[:], rows_dlT[:],
                             start=True, stop=True)
            # er channels directly into fbu3; dl to scratch
            nc.vector.tensor_scalar(fbu_v4[:, 0],
                                    ps_cols_er.rearrange("w (n h) -> w n h", n=4),
                                    cnt_sb, None, op0=ALU.is_equal)
            nc.vector.tensor_scalar(dl_t[:], ps_cols_dl[:], 0.5, None,
                                    op0=ALU.is_gt)

        def morph_stage_2():
            # bg = (dl == 0), maskd = dl - er
            dl_v = dl_t.rearrange("w (n h) -> w n h", n=4)
            nc.vector.tensor_scalar(fbu_v4[:, 1], dl_v, 0.0, None,
                                    op0=ALU.is_equal)
            nc.vector.tensor_tensor(fbu_v4[:, 2], dl_v, fbu_v4[:, 0],
                                    op=ALU.subtract)

        morph_stages = {0: morph_stage_0, 1: morph_stage_1, 2: morph_stage_2}

        # ---------------- pass 1: feature transpose + mid -------------------
        # mid matmuls are deferred by DEFER groups: they depend on fbu3, and
        # a stalled mid would block every later (in-order) PE instruction.
        mid_r = persist.tile([12, 256], F32R)
        with tc.tile_pool(name="mid_ps", bufs=1, space="PSUM") as midps, \
             tc.tile_pool(name="p1_ps", bufs=2, space="PSUM") as p1ps, \
             tc.tile_pool(name="p1_sb", bufs=8) as p1sb:
            mid_ps = midps.tile([12, 256], F32)

            def emit_mid(g, ft):
                for j in range(4):
                    h = 4 * g + j
                    nc.tensor.matmul(mid_ps[:], fbu_v12[:, :, h],
                                     ft[:, j * 256:(j + 1) * 256],
                                     start=(h == 0), stop=(h == P - 1),
                                     skip_group_check=True)

            DEFER = 6
            pending = []
            for g in range(P // 4):
                tr = p1ps.tile([P, 1024], F32R, tag="tr")
                for j in range(4):
                    h = 4 * g + j
                    nc.tensor.matmul(tr[:, j * 256:j * 256 + P],
                                     feat[:, h * P:(h + 1) * P],
                                     ident[:], is_transpose=True)
                    nc.tensor.matmul(tr[:, j * 256 + P:(j + 1) * 256],
                                     feat[:, HW + h * P:HW + (h + 1) * P],
                                     ident[:], is_transpose=True)
                ft = p1sb.tile([P, 1024], F32R, tag="ft")
                nc.vector.tensor_copy(ft[:, 0:512], tr[:, 0:512])
                nc.scalar.copy(ft[:, 512:1024], tr[:, 512:1024])
                if g in morph_stages:
                    morph_stages[g]()
                pending.append((g, ft))
                if g >= DEFER:
                    emit_mid(*pending.pop(0))
                    if len(pending) > 3:
                        emit_mid(*pending.pop(0))
                if g < 8:
                    filler(2)
                elif g < 16:
                    filler(1)
            for p in pending:
                emit_mid(*p)
            nc.vector.tensor_copy(mid_r[:], mid_ps[:])
        morph_ctx.close()

        # ---------------- small stage: mid^T, A'^T, G^T, Wc, u --------------
        with tc.tile_pool(name="sm_ps", bufs=1, space="PSUM") as smps, \
             tc.tile_pool(name="sm_sb", bufs=1) as smsb:
            # mid^T via PE transpose of [12,128] chunks (fp32r)
            ps_mt = smps.tile([P, 24], F32R, tag="mt")
            for ci in range(2):
                nc.tensor.matmul(ps_mt[:, ci * 12:(ci + 1) * 12],
                                 mid_r[:, ci * P:(ci + 1) * P],
                                 ident[0:12, 0:12], is_transpose=True)
            mid_t = smsb.tile([P, 24], F32R)
            nc.vector.tensor_copy(mid_t[:], ps_mt[:])
            filler(2)

            # A'^T = mid @ Wo2'^T   [12, 256]
            ps_at = smps.tile([12, 256], F32, tag="at")
            nc.tensor.matmul(ps_at[:], mid_t[:, 0:12], wo2_sb[:, 0:256],
                             start=True, stop=False)
            nc.tensor.matmul(ps_at[:], mid_t[:, 12:24], wo2_sb[:, 256:512],
                             start=False, stop=True)
            a_t = smsb.tile([12, 256], F32R)
            nc.scalar.copy(a_t[:], ps_at[:])
            filler(2)

            # G^T[c, o] = sum_k mid[k, c] A'^T[k, o];  chunks ci on partitions
            ps_gt = smps.tile([P, 512], F32, tag="gt")
            for ci in range(2):
                nc.tensor.matmul(ps_gt[:, ci * 256:(ci + 1) * 256],
                                 mid_r[:, ci * P:(ci + 1) * P], a_t[:],
                                 start=True, stop=True)
            gt_r = smsb.tile([P, 512], F32R)
            nc.vector.tensor_copy(gt_r[:], ps_gt[:])
            gt_f = smsb.tile([P, 512], F32)
            nc.scalar.copy(gt_f[:], ps_gt[:])
            filler(2)

            # X = Wf'^T @ G^T (= (G Wf')^T); blocks a (c_in chunk) on parts
            ps_x = smps.tile([P, 512], F32, tag="x")
            for a in range(2):
                for ci in range(2):
                    nc.tensor.matmul(ps_x[:, a * 256:(a + 1) * 256],
                                     wfn_sb[:, (ci * 2 + a) * P:(ci * 2 + a + 1) * P],
                                     gt_r[:, ci * 256:(ci + 1) * 256],
                                     start=(ci == 0), stop=(ci == 1),
                                     skip_group_check=True)

            # u = G @ beta_f + beta_o  per o-blk (fp32 matmuls: fp32r with
            # free size 1 violates the s3d3 ISA restrictions)
            ps_u = smps.tile([P, 2], F32, tag="u")
            for o in range(2):
                nc.tensor.matmul(ps_u[:, o:o + 1],
                                 gt_f[:, o * P:(o + 1) * P],
                                 betaf_sb[:, 0:1], start=True, stop=False,
                                 skip_group_check=True)
                nc.tensor.matmul(ps_u[:, o:o + 1],
                                 gt_f[:, 256 + o * P:256 + (o + 1) * P],
                                 betaf_sb[:, 1:2], start=False, stop=True,
                                 skip_group_check=True)
            nc.vector.tensor_tensor(u_sb[:], ps_u[:], betao_sb, op=ALU.add)

            # Wc^T = Wo1'^T + X  (blocks (a*2+o) align with [a*256 + o*128])
            nc.vector.tensor_tensor(wc_sb[:], ps_x[:], wo1_sb, op=ALU.add)

        fill_ctx.close()

        # ---------------- pass 2: out = Wc @ f + u --------------------------
        # store-bound; per pair of 512-col tiles: 4 matmuls per o-block into
        # a [128,1024] PSUM tile, evicted whole (DVE: rows 0:128, Act: rows
        # 128:256).  Stores are grouped to 2048 cols in the bulk (few
        # dma_starts keep the queues at full bandwidth), 512/1024 at the
        # head and tail.
        with tc.tile_pool(name="p2_ps", bufs=2, space="PSUM") as p2ps, \
             tc.tile_pool(name="p2_sb", bufs=2) as p2sb:
            STORE_GROUPS = [1, 1, 2, 2, 2, 2, 2, 2, 1, 1]  # units of 1024 cols
            pair = 0
            for npair in STORE_GROUPS:
                ot0 = p2sb.tile([P, 2048], F32, tag="ot0")
                ot1 = p2sb.tile([P, 2048], F32, tag="ot1")
                base = pair * 1024
                for k in range(npair):
                    po0 = p2ps.tile([P, 1024], F32, tag="po0")
                    po1 = p2ps.tile([P, 1024], F32, tag="po1")
                    first = (pair == 0 and k == 0)
                    for tt in range(2):
                        cc = (pair + k) * 1024 + tt * 512
                        for o, po in ((0, po0), (1, po1)):
                            ops = po[:, tt * 512:(tt + 1) * 512]
                            nc.tensor.matmul(ops,
                                             wc_sb[:, o * P:(o + 1) * P],
                                             feat[:, cc:cc + 512],
                                             start=True, stop=False,
                                             skip_group_check=True)
                            nc.tensor.matmul(ops,
                                             wc_sb[:, (2 + o) * P:(3 + o) * P],
                                             feat[:, HW + cc:HW + cc + 512],
                                             start=False, stop=True,
                                             skip_group_check=True)
                        if first:
                            # very first 512-col halves: evict + store
                            # immediately so the store pipe starts early
                            sl = slice(tt * 512, (tt + 1) * 512)
                            nc.vector.tensor_scalar(ot0[:, sl], po0[:, sl],
                                                    u_sb[:, 0:1], None,
                                                    op0=ALU.add)
                            nc.scalar.activation(ot1[:, sl], po1[:, sl],
                                                 ACTF.Identity,
                                                 bias=u_sb[:, 1:2])
                            cs = cc
                            nc.sync.dma_start(out=out[0:P, cs:cs + 512],
                                              in_=ot0[:, sl])
                            nc.gpsimd.dma_start(out=out[P:C, cs:cs + 512],
                                                in_=ot1[:, sl])
                    if not first:
                        kk = slice(k * 1024, (k + 1) * 1024)
                        nc.vector.tensor_scalar(ot0[:, kk], po0[:], u_sb[:, 0:1],
                                                None, op0=ALU.add)
                        nc.scalar.activation(ot1[:, kk], po1[:], ACTF.Identity,
                                             bias=u_sb[:, 1:2])
                if pair != 0:
                    w2 = npair * 1024
                    nc.sync.dma_start(out=out[0:P, base:base + w2],
                                      in_=ot0[:, 0:w2])
                    nc.gpsimd.dma_start(out=out[P:C, base:base + w2],
                                        in_=ot1[:, 0:w2])
                pair += npair

    nc.compile()
    _NC_CACHE["nc"] = nc
    return nc


def prepare_in_maps(feature, m, W_f, g_f, b_f, mu_f, v_f, W_o, g_o, b_o, mu_o, v_o):
    feature = np.asarray(feature, dtype=np.float32)
    m = np.asarray(m, dtype=np.float32)
    W_f = np.asarray(W_f, dtype=np.float32)
    W_o = np.asarray(W_o, dtype=np.float32)
    g_f, b_f, mu_f, v_f = (np.asarray(x, dtype=np.float32) for x in (g_f, b_f, mu_f, v_f))
    g_o, b_o, mu_o, v_o = (np.asarray(x, dtype=np.float32) for x in (g_o, b_o, mu_o, v_o))

    inv_f = g_f / np.sqrt(v_f + EPS)
    beta_f_v = b_f - mu_f * inv_f
    inv_o = g_o / np.sqrt(v_o + EPS)
    beta_o_v = b_o - mu_o * inv_o
    Wf_p = (inv_f[:, None] * W_f).astype(np.float32)          # [C, C]
    Wo1_p = (inv_o[:, None] * W_o[:, :C]).astype(np.float32)  # [C, C]
    Wo2_p = (inv_o[:, None] * W_o[:, C:]).astype(np.float32)  # [C, C]

    def blocks_t(Wp):
        # lhsT layout: blocks ci*2+o of Wp^T
        a = np.empty((P, 512), np.float32)
        for ci in range(2):
            for o in range(2):
                a[:, (ci * 2 + o) * P:(ci * 2 + o + 1) * P] = \
                    Wp[o * P:(o + 1) * P, ci * P:(ci + 1) * P].T
        return a

    def blocks_n(Wp):
        # natural-layout blocks ci*2+a: Wp[ci*128:(ci+1)*128, a*128:(a+1)*128]
        a_ = np.empty((P, 512), np.float32)
        for ci in range(2):
            for a in range(2):
                a_[:, (ci * 2 + a) * P:(ci * 2 + a + 1) * P] = \
                    Wp[ci * P:(ci + 1) * P, a * P:(a + 1) * P]
        return a_

    idx = np.arange(P)
    band_er_np = ((idx[:, None] >= idx[None, :] - 8) &
                  (idx[:, None] <= idx[None, :] + 4)).astype(np.float32)
    cnt_er = band_er_np.sum(axis=0, dtype=np.float32).reshape(P, 1)

    pkr = np.empty((P, 1536), np.float32)
    pkr[:, 0:512] = np.concatenate([Wo2_p.T[0:P, :], Wo2_p.T[P:C, :]], axis=1)
    pkr[:, 512:1024] = blocks_n(Wf_p)
    pkr[:, 1024:1536] = blocks_t(Wo1_p)

    pkf = np.empty((P, 517), np.float32)
    pkf[:, 512:514] = beta_o_v.reshape(2, P).T
    pkf[:, 514:516] = beta_f_v.reshape(2, P).T
    pkf[:, 516:517] = cnt_er

    in_maps = []
    for b in range(B):
        im = {"pkr": pkr}
        pkf_b = pkf.copy()
        # m per class into columns [n*128:(n+1)*128]
        pkf_b[:, 0:512] = np.transpose(m[b], (1, 0, 2)).reshape(P, 512)
        im["pkf"] = pkf_b
        im["feature"] = np.ascontiguousarray(feature[b].reshape(C, HW))
        in_maps.append(im)
    return in_maps


def kernel(feature, m, W_f, g_f, b_f, mu_f, v_f, W_o, g_o, b_o, mu_o, v_o):
    nc = build()
    in_maps = prepare_in_maps(feature, m, W_f, g_f, b_f, mu_f, v_f,
                              W_o, g_o, b_o, mu_o, v_o)
    res = bass_utils.run_bass_kernel_spmd(nc, in_maps, list(range(B)))
    out = np.empty((B, C, H, W), np.float32)
    for b in range(B):
        out[b] = res.results[b]["out"].reshape(C, H, W)
    return out


# revision 32
# speedup vs baseline: 1.1194x; 1.1194x over previous
"""Trainium2 Bass kernel for nn_BDFM_Multi (B=8,C=256,H=W=128,N=4).

Data-parallel over batch: one batch element per NeuronCore (8 cores).

Per-core computation (feature f [C,HW], m [N,H,W], HW=16384):
  z    = (m > 0.3)                                  binary
  er   = 13-tap separable min-filter(z), dl = 13-tap separable max-filter(z)
         (composition of 4 iters of 4x4 cv2-style erode/dilate)
         -> computed as banded 0/1 matmuls + thresholds (exact on binary data)
  fbu  = per-class channels (er, 1-dl, dl-er)       [12, HW]
  mid  = fbu @ f^T                                  [12, C]
  A'   = Wo2' @ mid^T                               [C, 12]
  G    = A' @ mid                                   [C, C]
  Wc   = Wo1' + G @ Wf'                             [C, C]  <- key collapse:
         out = Wo1'@f + G@(Wf'@f + beta_f 1^T) + beta_o 1^T
             = Wc @ f + u 1^T,   u = G @ beta_f + beta_o
  out  = Wc @ f + u                                 [C, HW]
  (exact algebraic refactor of out = BN(Wo @ [f; mid^T @ (mid @ BN(Wf@f))]))

Schedule notes (the kernel is HBM-bound: 16MB in + 16MB out per core with a
true barrier between them, since every output element depends on all of mid):
  - the whole feature load rides the SP (sync) HWDGE queue, which is alive
    ~6us before the SWDGE path; blk0/blk1 chunk pairs are interleaved with
    small chunks first (early pass-1 start) and last (small tail residual).
  - band/identity constants are generated on-chip (affine_select), not DMA'd.
  - pass 1 (PE transposes + mid accumulation) is software-pipelined and its
    PSUM evictions are split DVE/Act per half-group so it tracks DMA rate.
  - pass 2 is store-bound; PSUM tiles per output-row-block are evicted in
    1024-col ops (DVE for rows 0:128, Act for 128:256) and stored per 1024
    cols on the SP + gpsimd queues.

All big matmuls run in float32r (~2-4e-4 rel err, full PE rate).
"""
import numpy as np
from contextlib import ExitStack

import concourse.bass as bass
import concourse.mybir as mybir
import concourse.tile as tile
from concourse import bacc
from concourse import bass_utils
from concourse.masks import make_identity

F32 = mybir.dt.float32
F32R = mybir.dt.float32r
ALU = mybir.AluOpType
ACTF = mybir.ActivationFunctionType

B, C, H, W, N = 8, 256, 128, 128, 4
HW = H * W
EPS = 1e-5
P = 128

# feature load chunk sizes (cols per block).  Few, large chunks keep the
# queues at full bandwidth (each dma_start boundary costs a gap); the two
# queues land each blk0/blk1 pair concurrently, and the p-state filler
# matmuls bridge the 4-group visibility quantum of a 2048-col chunk.
# 512-col chunks at the head (fast pass-1 start) and tail (small residual).
CHUNKS = [512, 512, 1024] + [2048] * 6 + [1024, 512, 512]
assert sum(CHUNKS) == HW

_NC_CACHE = {}


def _band(nc, t, lo, hi):
    # t[i, m] = 1.0 iff m - lo <= i <= m + hi  (both affine_selects keep
    # where the affine expr passes, else fill 0)
    nc.gpsimd.memset(t, 1.0)
    nc.gpsimd.affine_select(out=t, in_=t, compare_op=ALU.is_ge, fill=0.0,
                            base=lo, pattern=[[-1, P]], channel_multiplier=1)
    # i <= m + hi  expressed as  (-i + m + hi) >= 0  (is_le unimplemented)
    nc.gpsimd.affine_select(out=t, in_=t, compare_op=ALU.is_ge, fill=0.0,
                            base=hi, pattern=[[1, P]], channel_multiplier=-1)


def build():
    if "nc" in _NC_CACHE:
        return _NC_CACHE["nc"]
    nc = bacc.Bacc(trn_type="TRN2", target_bir_lowering=False, debug=False)

    feature = nc.dram_tensor("feature", [C, HW], F32R, kind="ExternalInput")
    pkr = nc.dram_tensor("pkr", [P, 1536], F32R, kind="ExternalInput")
    pkf = nc.dram_tensor("pkf", [P, 517], F32, kind="ExternalInput")
    out = nc.dram_tensor("out", [C, HW], F32, kind="ExternalOutput")

    with tile.TileContext(nc) as tc, ExitStack() as ctx:
        persist = ctx.enter_context(tc.tile_pool(name="persist", bufs=1))

        feat = persist.tile([P, 2 * HW], F32R)     # c-blk0 cols | c-blk1 cols
        pkr_sb = persist.tile([P, 1536], F32R)
        pkf_sb = persist.tile([P, 517], F32)
        ident = persist.tile([P, P], F32R)
        band_er = persist.tile([P, P], F32R)
        band_dl = persist.tile([P, P], F32R)
        cgen = persist.tile([P, 3 * P], F32)       # Pool-written const scratch
        fbuT = persist.tile([P, 12 * P], F32R)     # [w, h*12 + k]
        wc_sb = persist.tile([P, 512], F32R)       # Wc^T blocks (a*2+o)
        u_sb = persist.tile([P, 2], F32)
        warm = persist.tile([P, 512], F32)
        warmb = persist.tile([P, 256], mybir.dt.bfloat16)  # p-state filler
        z_sb = persist.tile([P, N * P], F32R)

        wo2_sb = pkr_sb[:, 0:512]
        wfn_sb = pkr_sb[:, 512:1024]
        wo1_sb = pkr_sb[:, 1024:1536]
        m_sb = pkf_sb[:, 0:512]
        betao_sb = pkf_sb[:, 512:514]
        betaf_sb = pkf_sb[:, 514:516]
        cnt_sb = pkf_sb[:, 516:517]

        # ---------------- loads: everything on the SP HWDGE queue ----------
        # feature/pkr DRAM tensors are declared float32r (same 4-byte IEEE
        # bits; the PE rounds internally), so these DMAs are cast-free and
        # can ride the SP HWDGE queue, alive ~6us before the SWDGE path.
        # ---------------- on-chip constants --------------------------------
        # Pool affine_select writes fp32 scratch (Pool cannot write fp32r);
        # DVE copies perform the fp32->fp32r rounding cast.
        make_identity(nc, cgen[:, 0:P])
        # erosion: output i covers input [i-8, i+4]; band_er[i,m]=1 iff
        # m-8 <= i <= m+4.  dilation: [i-4, i+8].
        _band(nc, cgen[:, P:2 * P], 8, 4)
        _band(nc, cgen[:, 2 * P:3 * P], 4, 8)
        nc.vector.tensor_copy(ident[:], cgen[:, 0:P])
        nc.vector.tensor_copy(band_er[:], cgen[:, P:2 * P])
        nc.vector.tensor_copy(band_dl[:], cgen[:, 2 * P:3 * P])

        # single SP HWDGE queue: measured ~383 GB/s; multi-queue splits and
        # many small dma_starts both measured slower.
        nc.sync.dma_start(out=pkf_sb[:], in_=pkf[:])
        c0 = 0
        for i, w in enumerate(CHUNKS):
            c1 = c0 + w
            nc.sync.dma_start(out=feat[:, c0:c1], in_=feature[0:P, c0:c1])
            nc.sync.dma_start(out=feat[:, HW + c0:HW + c1],
                              in_=feature[P:C, c0:c1])
            c0 = c1
        # params for the small stage; needed only at ~load end
        nc.sync.dma_start(out=pkr_sb[:], in_=pkr[:])

        # ---------------- warmup + morphology ------------------------------
        with tc.tile_pool(name="warm_ps", bufs=1, space="PSUM") as wps:
            nc.vector.memset(warm[:], 0.0)
            nc.vector.memset(warmb[:], 0.0)
            # trigger the one-time ACT table load early (off critical path)
            nc.scalar.copy(u_sb[:, 0:1], warm[:, 0:1])
            wp = wps.tile([P, 512], F32)
            for _ in range(2):
                nc.tensor.matmul(wp[:, 0:P], warm[:, 0:P], warm[:, 0:P],
                                 start=True, stop=True, skip_group_check=True)

        # ---------------- morphology + pass 1, interleaved ------------------
        # Engines execute in issue order, so morphology ops are interleaved
        # into the first pass-1 groups: otherwise the early pass-1 PSUM
        # evictions queue behind the whole morph chain on DVE/Act, the
        # 2-buffer transpose pool backs up, and the (in-order) PE stalls.
        # fbu is built as three contiguous [128,512] planes (er|bg|maskd) in
        # one tile; the mid matmul reads a strided [w, n, j] lhsT AP.
        fill_ctx = ExitStack()
        fillps = fill_ctx.enter_context(
            tc.tile_pool(name="fill_ps", bufs=1, space="PSUM"))
        fill_ps = fillps.tile([P, 256], F32)

        def filler(k):
            for _ in range(k):
                nc.tensor.matmul(fill_ps[:], warmb[:, 0:P], warmb[:],
                                 start=True, stop=True,
                                 skip_group_check=True)

        morph_ctx = ExitStack()
        mo = morph_ctx.enter_context(tc.tile_pool(name="morph", bufs=1))
        mops = morph_ctx.enter_context(tc.tile_pool(name="morph_ps", bufs=2,
                                                    space="PSUM"))
        fbu3 = persist.tile([P, 3 * 512], F32R)    # col = (n*3+j)*128 + h
        dl_t = persist.tile([P, 512], F32)

        nc.vector.tensor_scalar(z_sb[:], m_sb, 0.3, None, op0=ALU.is_gt)
        ps_rows_er = mops.tile([P, N * P], F32, tag="mps")
        nc.tensor.matmul(ps_rows_er[:], band_er[:], z_sb[:],
                         start=True, stop=True)
        ps_rows_dl = mops.tile([P, N * P], F32, tag="mps")
        nc.tensor.matmul(ps_rows_dl[:], band_dl[:], z_sb[:],
                         start=True, stop=True)
        rows_er = mo.tile([P, N * P], F32R)
        nc.vector.tensor_scalar(rows_er[:], ps_rows_er[:], cnt_sb, None,
                                op0=ALU.is_equal)
        rows_dl = mo.tile([P, N * P], F32R)
        nc.vector.tensor_scalar(rows_dl[:], ps_rows_dl[:], 0.5, None,
                                op0=ALU.is_gt)
        rows_erT = mo.tile([P, N * P], F32R)
        rows_dlT = mo.tile([P, N * P], F32R)

        def morph_stage_0():
            # transpose each class tile -> [w, h]; one big eviction per kind
            ps_trs = []
            for n in range(N):
                ps_tr = mops.tile([P, 2 * P], F32R, tag="mps", name=f"ps_tr{n}")
                nc.tensor.matmul(ps_tr[:, 0:P], rows_er[:, n * P:(n + 1) * P],
                                 ident[:], is_transpose=True)
                nc.tensor.matmul(ps_tr[:, P:2 * P],
                                 rows_dl[:, n * P:(n + 1) * P],
                                 ident[:], is_transpose=True)
                ps_trs.append(ps_tr)
            for n, ps_tr in enumerate(ps_trs):
                nc.scalar.copy(rows_erT[:, n * P:(n + 1) * P], ps_tr[:, 0:P])
                nc.scalar.copy(rows_dlT[:, n * P:(n + 1) * P], ps_tr[:, P:2 * P])

        # fbu3 channel-plane views: v4[:, j] = [w, n, h] (k = n*3 + j)
        fbu_v4 = fbu3.rearrange("w (n j h) -> w j n h", n=4, j=3)
        fbu_v12 = fbu3.rearrange("w (k h) -> w k h", k=12)

        def morph_stage_1():
            ps_cols_er = mops.tile([P, N * P], F32, tag="mps")
            nc.tensor.matmul(ps_cols_er[:], band_er[:], rows_erT[:],
                             start=True, stop=True)
            ps_cols_dl = mops.tile([P, N * P], F32, tag="mps")
            nc.tensor.matmul(ps_cols_dl[:], band_dl[:], rows_dlT[:],
                             start=True, stop=True)
            # er channels directly into fbu3; dl to scratch
            nc.vector.tensor_scalar(fbu_v4[:, 0],
                                    ps_cols_er.rearrange("w (n h) -> w n h", n=4),
                                    cnt_sb, None, op0=ALU.is_equal)
            nc.vector.tensor_scalar(dl_t[:], ps_cols_dl[:], 0.5, None,
                                    op0=ALU.is_gt)

        def morph_stage_2():
            # bg = (dl == 0), maskd = dl - er
            dl_v = dl_t.rearrange("w (n h) -> w n h", n=4)
            nc.vector.tensor_scalar(fbu_v4[:, 1], dl_v, 0.0, None,
                                    op0=ALU.is_equal)
            nc.vector.tensor_tensor(fbu_v4[:, 2], dl_v, fbu_v4[:, 0],
                                    op=ALU.subtract)

        morph_stages = {0: morph_stage_0, 1: morph_stage_1, 2: morph_stage_2}

        # ---------------- pass 1: feature transpose + mid -------------------
        # mid matmuls are deferred by DEFER groups: they depend on fbu3, and
        # a stalled mid would block every later (in-order) PE instruction.
        mid_r = persist.tile([12, 256], F32R)
        with tc.tile_pool(name="mid_ps", bufs=1, space="PSUM") as midps, \
             tc.tile_pool(name="p1_ps", bufs=2, space="PSUM") as p1ps, \
             tc.tile_pool(name="p1_sb", bufs=8) as p1sb:
            mid_ps = midps.tile([12, 256], F32)

            def emit_mid(g, ft):
                for j in range(4):
                    h = 4 * g + j
                    nc.tensor.matmul(mid_ps[:], fbu_v12[:, :, h],
                                     ft[:, j * 256:(j + 1) * 256],
                                     start=(h == 0), stop=(h == P - 1),
                                     skip_group_check=True)

            DEFER = 6
            pending = []
            for g in range(P // 4):
                tr = p1ps.tile([P, 1024], F32R, tag="tr")
                for j in range(4):
                    h = 4 * g + j
                    nc.tensor.matmul(tr[:, j * 256:j * 256 + P],
                                     feat[:, h * P:(h + 1) * P],
                                     ident[:], is_transpose=True)
                    nc.tensor.matmul(tr[:, j * 256 + P:(j + 1) * 256],
                                     feat[:, HW + h * P:HW + (h + 1) * P],
                                     ident[:], is_transpose=True)
                ft = p1sb.tile([P, 1024], F32R, tag="ft")
                nc.vector.tensor_copy(ft[:, 0:512], tr[:, 0:512])
                nc.scalar.copy(ft[:, 512:1024], tr[:, 512:1024])
                if g in morph_stages:
                    morph_stages[g]()
                pending.append((g, ft))
                if g >= DEFER:
                    emit_mid(*pending.pop(0))
                    if len(pending) > 3:
                        emit_mid(*pending.pop(0))
                if g < 8:
                    filler(2)
                elif g < 16:
                    filler(1)
            for p in pending:
                emit_mid(*p)
            nc.vector.tensor_copy(mid_r[:], mid_ps[:])
        morph_ctx.close()

        # ---------------- small stage: mid^T, A'^T, G^T, Wc, u --------------
        with tc.tile_pool(name="sm_ps", bufs=1, space="PSUM") as smps, \
             tc.tile_pool(name="sm_sb", bufs=1) as smsb:
            # mid^T via PE transpose of [12,128] chunks (fp32r)
            ps_mt = smps.tile([P, 24], F32R, tag="mt")
            for ci in range(2):
                nc.tensor.matmul(ps_mt[:, ci * 12:(ci + 1) * 12],
                                 mid_r[:, ci * P:(ci + 1) * P],
                                 ident[0:12, 0:12], is_transpose=True)
            mid_t = smsb.tile([P, 24], F32R)
            nc.vector.tensor_copy(mid_t[:], ps_mt[:])
            filler(2)

            # A'^T = mid @ Wo2'^T   [12, 256]
            ps_at = smps.tile([12, 256], F32, tag="at")
            nc.tensor.matmul(ps_at[:], mid_t[:, 0:12], wo2_sb[:, 0:256],
                             start=True, stop=False)
            nc.tensor.matmul(ps_at[:], mid_t[:, 12:24], wo2_sb[:, 256:512],
                             start=False, stop=True)
            a_t = smsb.tile([12, 256], F32R)
            nc.scalar.copy(a_t[:], ps_at[:])
            filler(2)

            # G^T[c, o] = sum_k mid[k, c] A'^T[k, o];  chunks ci on partitions
            ps_gt = smps.tile([P, 512], F32, tag="gt")
            for ci in range(2):
                nc.tensor.matmul(ps_gt[:, ci * 256:(ci + 1) * 256],
                                 mid_r[:, ci * P:(ci + 1) * P], a_t[:],
                                 start=True, stop=True)
            gt_r = smsb.tile([P, 512], F32R)
            nc.vector.tensor_copy(gt_r[:], ps_gt[:])
            gt_f = smsb.tile([P, 512], F32)
            nc.scalar.copy(gt_f[:], ps_gt[:])
            filler(2)

            # X = Wf'^T @ G^T (= (G Wf')^T); blocks a (c_in chunk) on parts
            ps_x = smps.tile([P, 512], F32, tag="x")
            for a in range(2):
                for ci in range(2):
                    nc.tensor.matmul(ps_x[:, a * 256:(a + 1) * 256],
                                     wfn_sb[:, (ci * 2 + a) * P:(ci * 2 + a + 1) * P],
                                     gt_r[:, ci * 256:(ci + 1) * 256],
                                     start=(ci == 0), stop=(ci == 1),
                                     skip_group_check=True)

            # u = G @ beta_f + beta_o  per o-blk (fp32 matmuls: fp32r with
            # free size 1 violates the s3d3 ISA restrictions)
            ps_u = smps.tile([P, 2], F32, tag="u")
            for o in range(2):
                nc.tensor.matmul(ps_u[:, o:o + 1],
                                 gt_f[:, o * P:(o + 1) * P],
                                 betaf_sb[:, 0:1], start=True, stop=False,
                                 skip_group_check=True)
                nc.tensor.matmul(ps_u[:, o:o + 1],
                                 gt_f[:, 256 + o * P:256 + (o + 1) * P],
                                 betaf_sb[:, 1:2], start=False, stop=True,
                                 skip_group_check=True)
            nc.vector.tensor_tensor(u_sb[:], ps_u[:], betao_sb, op=ALU.add)

            # Wc^T = Wo1'^T + X  (blocks (a*2+o) align with [a*256 + o*128])
            nc.vector.tensor_tensor(wc_sb[:], ps_x[:], wo1_sb, op=ALU.add)

        fill_ctx.close()

        # ---------------- pass 2: out = Wc @ f + u --------------------------
        # store-bound; per pair of 512-col tiles: 4 matmuls per o-block into
        # a [128,1024] PSUM tile, evicted whole (DVE: rows 0:128, Act: rows
        # 128:256).  Stores are grouped to 2048 cols in the bulk (few
        # dma_starts keep the queues at full bandwidth), 512/1024 at the
        # head and tail.
        with tc.tile_pool(name="p2_ps", bufs=2, space="PSUM") as p2ps, \
             tc.tile_pool(name="p2_sb", bufs=2) as p2sb:
            STORE_GROUPS = [1, 1, 2, 2, 2, 2, 2, 2, 1, 1]  # units of 1024 cols
            pair = 0
            for npair in STORE_GROUPS:
                ot0 = p2sb.tile([P, 2048], F32, tag="ot0")
                ot1 = p2sb.tile([P, 2048], F32, tag="ot1")
                base = pair * 1024
                for k in range(npair):
                    po0 = p2ps.tile([P, 1024], F32, tag="po0")
                    po1 = p2ps.tile([P, 1024], F32, tag="po1")
                    first = (pair == 0 and k == 0)
                    for tt in range(2):
                        cc = (pair + k) * 1024 + tt * 512
                        for o, po in ((0, po0), (1, po1)):
                            ops = po[:, tt * 512:(tt + 1) * 512]
                            nc.tensor.matmul(ops,
                                             wc_sb[:, o * P:(o + 1) * P],
                                             feat[:, cc:cc + 512],
                                             start=True, stop=False,
                                             skip_group_check=True)
                            nc.tensor.matmul(ops,
                                             wc_sb[:, (2 + o) * P:(3 + o) * P],
                                             feat[:, HW + cc:HW + cc + 512],
                                             start=False, stop=True,
                                             skip_group_check=True)
                        if first:
                            # very first 512-col halves: evict + store
                            # immediately so the store pipe starts early
                            sl = slice(tt * 512, (tt + 1) * 512)
                            nc.vector.tensor_scalar(ot0[:, sl], po0[:, sl],
                                                    u_sb[:, 0:1], None,
                                                    op0=ALU.add)
                            nc.scalar.activation(ot1[:, sl], po1[:, sl],
                                                 ACTF.Identity,
                                                 bias=u_sb[:, 1:2])
                            cs = cc
                            nc.sync.dma_start(out=out[0:P, cs:cs + 512],
                                              in_=ot0[:, sl])
                            nc.gpsimd.dma_start(out=out[P:C, cs:cs + 512],
                                                in_=ot1[:, sl])
                    if not first:
                        kk = slice(k * 1024, (k + 1) * 1024)
                        nc.vector.tensor_scalar(ot0[:, kk], po0[:], u_sb[:, 0:1],
                                                None, op0=ALU.add)
                        nc.scalar.activation(ot1[:, kk], po1[:], ACTF.Identity,
                                             bias=u_sb[:, 1:2])
                if pair != 0:
                    w2 = npair * 1024
                    nc.sync.dma_start(out=out[0:P, base:base + w2],
                                      in_=ot0[:, 0:w2])
                    nc.gpsimd.dma_start(out=out[P:C, base:base + w2],
                                        in_=ot1[:, 0:w2])
                pair += npair

    nc.compile()
    _NC_CACHE["nc"] = nc
    return nc


def prepare_in_maps(feature, m, W_f, g_f, b_f, mu_f, v_f, W_o, g_o, b_o, mu_o, v_o):
    feature = np.asarray(feature, dtype=np.float32)
    m = np.asarray(m, dtype=np.float32)
    W_f = np.asarray(W_f, dtype=np.float32)
    W_o = np.asarray(W_o, dtype=np.float32)
    g_f, b_f, mu_f, v_f = (np.asarray(x, dtype=np.float32) for x in (g_f, b_f, mu_f, v_f))
    g_o, b_o, mu_o, v_o = (np.asarray(x, dtype=np.float32) for x in (g_o, b_o, mu_o, v_o))

    inv_f = g_f / np.sqrt(v_f + EPS)
    beta_f_v = b_f - mu_f * inv_f
    inv_o = g_o / np.sqrt(v_o + EPS)
    beta_o_v = b_o - mu_o * inv_o
    Wf_p = (inv_f[:, None] * W_f).astype(np.float32)          # [C, C]
    Wo1_p = (inv_o[:, None] * W_o[:, :C]).astype(np.float32)  # [C, C]
    Wo2_p = (inv_o[:, None] * W_o[:, C:]).astype(np.float32)  # [C, C]

    def blocks_t(Wp):
        # lhsT layout: blocks ci*2+o of Wp^T
        a = np.empty((P, 512), np.float32)
        for ci in range(2):
            for o in range(2):
                a[:, (ci * 2 + o) * P:(ci * 2 + o + 1) * P] = \
                    Wp[o * P:(o + 1) * P, ci * P:(ci + 1) * P].T
        return a

    def blocks_n(Wp):
        # natural-layout blocks ci*2+a: Wp[ci*128:(ci+1)*128, a*128:(a+1)*128]
        a_ = np.empty((P, 512), np.float32)
        for ci in range(2):
            for a in range(2):
                a_[:, (ci * 2 + a) * P:(ci * 2 + a + 1) * P] = \
                    Wp[ci * P:(ci + 1) * P, a * P:(a + 1) * P]
        return a_

    idx = np.arange(P)
    band_er_np = ((idx[:, None] >= idx[None, :] - 8) &
                  (idx[:, None] <= idx[None, :] + 4)).astype(np.float32)
    cnt_er = band_er_np.sum(axis=0, dtype=np.float32).reshape(P, 1)

    pkr = np.empty((P, 1536), np.float32)
    pkr[:, 0:512] = np.concatenate([Wo2_p.T[0:P, :], Wo2_p.T[P:C, :]], axis=1)
    pkr[:, 512:1024] = blocks_n(Wf_p)
    pkr[:, 1024:1536] = blocks_t(Wo1_p)

    pkf = np.empty((P, 517), np.float32)
    pkf[:, 512:514] = beta_o_v.reshape(2, P).T
    pkf[:, 514:516] = beta_f_v.reshape(2, P).T
    pkf[:, 516:517] = cnt_er

    in_maps = []
    for b in range(B):
        im = {"pkr": pkr}
        pkf_b = pkf.copy()
        # m per class into columns [n*128:(n+1)*128]
        pkf_b[:, 0:512] = np.transpose(m[b], (1, 0, 2)).reshape(P, 512)
        im["pkf"] = pkf_b
        im["feature"] = np.ascontiguousarray(feature[b].reshape(C, HW))
        in_maps.append(im)
    return in_maps


def kernel(feature, m, W_f, g_f, b_f, mu_f, v_f, W_o, g_o, b_o, mu_o, v_o):
    nc = build()
    in_maps = prepare_in_maps(feature, m, W_f, g_f, b_f, mu_f, v_f,
                              W_o, g_o, b_o, mu_o, v_o)
    res = bass_utils.run_bass_kernel_spmd(nc, in_maps, list(range(B)))
    out = np.empty((B, C, H, W), np.float32)
    for b in range(B):
        out[b] = res.results[b]["out"].reshape(C, H, W)
    return out
